# revision 2
# baseline (speedup 1.0000x reference)
"""Trainium2 Bass kernel for nn_LogicConv3d (differentiable-logic conv tree).

Problem (hardcoded): x [16,64,32,32] f32; idx_a/idx_b [64,900,64,3] i32;
w0..w6 [s,64,16] f32 (s = 64,32,16,8,4,2,1). Output [16,64,900,1] f32.

Math: per (kernel k, window p): gather 64 (a,b) leaf pairs from x, blend each
pair with soft-gate coefficients (softmax(w) @ GATE_M), then 6 more pairwise
tree levels.  mix(a,b) = c0 + c1*a + c2*b + c3*a*b.

v2 design (fp16 end-to-end):
 - F-sharding: core i handles batches (2i, 2i+1); pure SPMD across 8 cores.
 - Host builds a 576-row fp16 crop table XS[(c,ha,wa), 1920]: row = the full
   30x30x2 (h,w,b-interleaved) crop of channel c at shift (ha,wa), COMPACT in
   the first 1800 elements (120 pad to make rows 3840 B = 15*256 for
   dma_gather).  Leaf gathers become row-gathers: one dma_gather per L0 op
   fetches both sides (256 int16 row-ids -> [128, 2, 1920] fp16 tile).
   Compact rows mean every compute operand is contiguous step-1 fp16, which
   enables the DVE 2x packed mode (the f32 strided baseline ran 1x).
 - Tree levels on-chip: partition dim = (node-msb, kernel), free = (hh,ww,b).
 - mix is 3 ops: ACT: p = sc*a + bi; DVE stt: q = (b + be) * p;
   DVE stt: r = (a * c1) + q; additive constants fold into the next level's
   scalars (tree edges are single-use), applied once on host (gamma).
"""
import numpy as np

B, C, H, W = 16, 64, 32, 32
K = 64
RF = 3
DEPTH = 6
S = 64
PW = 30            # windows per axis
P = PW * PW        # 900
NCORES = 8
B2 = 2             # batches per core
F = P * B2         # free size (windows x batches) = 1800
NROW = C * RF * RF          # 576 crop-table rows
ROWE = 1920                 # padded row length (3840 B, mult of 256)
NQ = 2                      # SWDGE queues for gather round-robin

GATE_M = np.array([
    [0, 0, 0, 0], [0, 0, 0, 1], [0, 1, 0, -1], [0, 1, 0, 0],
    [0, 0, 1, -1], [0, 0, 1, 0], [0, 1, 1, -2], [0, 1, 1, -1],
    [1, -1, -1, 1], [1, -1, -1, 2], [1, 0, -1, 0], [1, 0, -1, 1],
    [1, -1, 0, 0], [1, -1, 0, 1], [1, 0, 0, -1], [1, 0, 0, 0],
], dtype=np.float32)  # [16 gates, 4] -> c0,c1,c2,c3 = GATE_M.T @ softmax(w)


# ---------------------------------------------------------------------------
# static schedule: the merge-tree op list (DFS order keeps live tiles small)
# ---------------------------------------------------------------------------
def _build_schedule():
    """Each mix op: dict(level, key, lanes, base, node[lanes], kern[lanes]).
    L0 ops gather their own leaves; level l>=1 ops read T_{l-1}[2k],[2k+1]."""
    ops = []

    def emit(l, key):
        if l == 0:
            lanes = np.arange(128)
            ops.append(dict(level=0, key=key, lanes=128, base=0,
                            node=key + 32 * (lanes >> 6), kern=lanes & 63))
            return
        emit(l - 1, 2 * key)
        emit(l - 1, 2 * key + 1)
        lanes = np.arange(128)
        nbits_out = 6 - l
        ops.append(dict(level=l, key=key, lanes=128, base=0,
                        node=((lanes >> 6) << (nbits_out - 1)) + key,
                        kern=lanes & 63))

    emit(4, 0)
    emit(4, 1)
    # L5: one full op; node i5 = lane>>6 (a DMA then realigns the top half
    # to a base-0 tile for L6's equal-base inputs)
    lanes = np.arange(128)
    ops.append(dict(level=5, key=0, lanes=128, base=0,
                    node=lanes >> 6, kern=lanes & 63))
    lanes = np.arange(64)
    ops.append(dict(level=6, key=0, lanes=64, base=0,
                    node=np.zeros(64, np.int64), kern=lanes))
    return ops


_SCHED = _build_schedule()
_NMIX = len(_SCHED)          # 64
_NCOLS = 4 * _NMIX + 4       # + final gamma column block


def _softmax_f32(w):
    w = w.astype(np.float64)
    m = w.max(-1, keepdims=True)
    e = np.exp(w - m)
    return e / e.sum(-1, keepdims=True)


def _coef_tables(ws):
    """ws = [w0..w6]. Returns coef matrix [128, _NCOLS] f32 with per-op scalar
    columns (sc, bias, beta, c1) and the final gamma column."""
    cs = []
    for wl in ws:
        p = _softmax_f32(wl)                      # [s, K, 16] f64
        cs.append(np.einsum('skg,gj->skj', p, GATE_M.astype(np.float64)))
    gamma = [None] * 7
    gamma[0] = cs[0][:, :, 0]                     # c0, alpha=0 at leaves
    for l in range(1, 7):
        gamma[l] = cs[l][:, :, 0] + cs[l][:, :, 1] * gamma[l - 1][0::2]
    coef = np.zeros((128, _NCOLS), dtype=np.float64)
    for i, op in enumerate(_SCHED):
        l, node, kern = op['level'], op['node'], op['kern']
        rows = op['base'] + np.arange(op['lanes'])
        c = cs[l][node, kern]                     # [lanes, 4]
        if l == 0:
            alpha = np.zeros(op['lanes'])
            beta = np.zeros(op['lanes'])
        else:
            alpha = gamma[l - 1][2 * node, kern]
            beta = gamma[l - 1][2 * node + 1, kern]
        coef[rows, 4 * i + 0] = c[:, 3]                      # ACT scale = c3
        coef[rows, 4 * i + 1] = c[:, 2] + alpha * c[:, 3]    # ACT bias
        coef[rows, 4 * i + 2] = beta                         # stt1 scalar
        coef[rows, 4 * i + 3] = c[:, 1]                      # stt2 scalar = c1
    coef[0:64, 4 * _NMIX] = gamma[6][0, :]                   # final add
    return coef.astype(np.float32)


def _row_tables(idx_a, idx_b):
    """dma_gather row-id tables [128, 16*32] int16 (only partitions 0..15
    carry data: idx for gathered vector i of op t lives at
    [i % 16, 16*t + i // 16]).  Row id = c*9 + ha*3 + wa."""
    tbl = np.zeros((128, 16 * 32), dtype=np.int16)
    for op in _SCHED:
        if op['level'] != 0:
            continue
        t = op['key']
        rcat = np.zeros(256, dtype=np.int64)
        for side, idx in ((0, idx_a), (1, idx_b)):
            ha = idx[op['kern'], 0, op['node'], 0].astype(np.int64)
            wa = idx[op['kern'], 0, op['node'], 1].astype(np.int64)
            ca = idx[op['kern'], 0, op['node'], 2].astype(np.int64)
            rcat[128 * side:128 * side + 128] = ca * 9 + ha * 3 + wa
        tbl[0:16, 16 * t:16 * t + 16] = rcat.reshape(16, 16).T
    return tbl


def _crop_table(xs):
    """xs: [C, H, W, B2] f32 b-interleaved slice -> XS [576, 1920] fp16."""
    XS = np.zeros((NROW, ROWE), dtype=np.float16)
    for ha in range(RF):
        for wa in range(RF):
            rows = np.arange(C) * 9 + ha * 3 + wa
            XS[rows, :F] = xs[:, ha:ha + PW, wa:wa + PW, :].reshape(
                C, F).astype(np.float16)
    return XS


# ---------------------------------------------------------------------------
# numpy emulator (mirrors the device schedule incl. fp16 rounding)
# ---------------------------------------------------------------------------
def _emulate_core(XS, rows, coef):
    """XS: [576,1920] fp16; rows: [128, 512] i16. Returns [64, F] f32."""
    f16 = np.float16
    tiles = {}
    for i, op in enumerate(_SCHED):
        l, key, n, base = op['level'], op['key'], op['lanes'], op['base']
        rws = base + np.arange(n)
        sc = coef[rws, 4 * i + 0][:, None].astype(np.float32)
        bi = coef[rws, 4 * i + 1][:, None].astype(np.float32)
        be = coef[rws, 4 * i + 2][:, None].astype(np.float32)
        c1 = coef[rws, 4 * i + 3][:, None].astype(np.float32)
        if l == 0:
            rcat = rows[0:16, 16 * key:16 * key + 16].T.reshape(-1)
            a = XS[rcat[0:128], :F].astype(np.float32)
            b = XS[rcat[128:256], :F].astype(np.float32)
        elif l < 5:
            a = tiles[(l - 1, 2 * key)].astype(np.float32)
            b = tiles[(l - 1, 2 * key + 1)].astype(np.float32)
        elif l == 5:
            a = tiles[(4, 0)].astype(np.float32)
            b = tiles[(4, 1)].astype(np.float32)
        else:
            a = tiles['T5'][0:64].astype(np.float32)
            b = tiles['T5'][64:128].astype(np.float32)
        p = f16(a * sc + bi).astype(np.float32)
        q = f16((b + be) * p).astype(np.float32)
        r = f16(a * c1 + q)
        if l == 5:
            tiles['T5'] = r
        else:
            tiles[(l, key)] = r
    return tiles[(6, 0)].astype(np.float32)


# ---------------------------------------------------------------------------
# Bass program (built once, cached)
# ---------------------------------------------------------------------------
_BASS_CACHE = {}


def _build_bass():
    if 'nc' in _BASS_CACHE:
        return _BASS_CACHE['nc']
    import concourse.bass as bass
    import concourse.mybir as mybir
    import concourse.tile as tile
    import concourse.bacc as bacc

    f32 = mybir.dt.float32
    f16 = mybir.dt.float16
    i16 = mybir.dt.int16
    nc = bacc.Bacc("TRN2", target_bir_lowering=False, debug=False,
                   num_devices=NCORES, num_swdge_queues=NQ)
    xs_d = nc.dram_tensor("xs", [NROW, ROWE], f16, kind="ExternalInput").ap()
    rows_d = nc.dram_tensor("rows", [128, 16 * 32], i16,
                            kind="ExternalInput").ap()
    coef_d = nc.dram_tensor("coef", [128, _NCOLS], f32,
                            kind="ExternalInput").ap()
    out_d = nc.dram_tensor("out", [64, F], f16, kind="ExternalOutput").ap()

    AL = mybir.AluOpType
    ACTF = mybir.ActivationFunctionType

    with tile.TileContext(nc) as tc:
        with (
            tc.tile_pool(name="const", bufs=1) as pc,
            tc.tile_pool(name="ab", bufs=6) as pab,
            tc.tile_pool(name="lvl", bufs=2) as plv,
            tc.tile_pool(name="t0p", bufs=2) as pt0,
            tc.tile_pool(name="tmp", bufs=6) as ptmp,
            tc.tile_pool(name="fin", bufs=1) as pfin,
        ):
            rows_t = pc.tile([128, 16 * 32], i16, tag="rows", name="rows_t")
            nc.sync.dma_start(rows_t[:], rows_d[:])
            coef_t = pc.tile([128, _NCOLS], f32, tag="coef", name="coef_t")
            nc.sync.dma_start(coef_t[:], coef_d[:])
            warm_t = pc.tile([1, 8], f32, tag="warm", name="warm_t")
            nc.scalar.activation(warm_t[:], coef_t[0:1, 0:8],
                                 ACTF.Identity, bias=0.0, scale=1.0)

            tiles = {}
            for i, op in enumerate(_SCHED):
                l, key, n, base = op['level'], op['key'], op['lanes'], op['base']
                sl = slice(base, base + n)
                sc = coef_t[sl, 4 * i + 0:4 * i + 1]
                bi = coef_t[sl, 4 * i + 1:4 * i + 2]
                be = coef_t[sl, 4 * i + 2:4 * i + 3]
                c1 = coef_t[sl, 4 * i + 3:4 * i + 4]
                if l == 0:
                    g_t = pab.tile([128, 2 * ROWE], f16, tag="AB", name="ab_t")
                    g_ap = g_t[:].rearrange("p (j e) -> p j e", j=2, e=ROWE)
                    nc.gpsimd.dma_gather(
                        g_ap, xs_d[:],
                        rows_t[:, 16 * key:16 * key + 16],
                        num_idxs=256, num_idxs_reg=256, elem_size=ROWE,
                        queue_num=key % NQ)
                    a_ap = g_t[:, 0:F]
                    b_ap = g_t[:, ROWE:ROWE + F]
                elif l < 5:
                    a_ap = tiles[(l - 1, 2 * key)][:, :]
                    b_ap = tiles[(l - 1, 2 * key + 1)][:, :]
                elif l == 5:
                    a_ap = tiles[(4, 0)][:, :]
                    b_ap = tiles[(4, 1)][:, :]
                else:
                    a_ap = tiles['T5'][0:64, :]
                    b_ap = tiles['T5b'][:, :]

                p_t = ptmp.tile([n, F], f16, tag="p", name="p")
                p_ap = p_t[:, :]
                nc.scalar.activation(p_ap, a_ap, ACTF.Identity,
                                     bias=bi, scale=sc)
                nc.vector.scalar_tensor_tensor(
                    out=p_ap, in0=b_ap, scalar=be, in1=p_ap,
                    op0=AL.add, op1=AL.mult)
                if l == 5:
                    r_t = pfin.tile([128, F], f16, tag="T5", name="t5")
                    tiles['T5'] = r_t
                elif l == 6:
                    r_t = pfin.tile([64, F], f16, tag="T6", name="t6")
                else:
                    pool = pt0 if l == 0 else plv
                    r_t = pool.tile([128, F], f16, tag=f"T{l}",
                                    name=f"t{l}_{key}")
                    tiles[(l, key)] = r_t
                if l == 6:
                    # compute + store in h-halves so the store of half 0
                    # overlaps the stt of half 1
                    for hh in (0, 1):
                        fs = slice(900 * hh, 900 * hh + 900)
                        nc.vector.scalar_tensor_tensor(
                            out=r_t[:, fs], in0=a_ap[:, fs], scalar=c1,
                            in1=p_ap[:, fs], op0=AL.mult, op1=AL.add)
                        nc.sync.dma_start(out_d[:, fs], r_t[:, fs])
                    continue
                nc.vector.scalar_tensor_tensor(
                    out=r_t[:, :], in0=a_ap, scalar=c1, in1=p_ap,
                    op0=AL.mult, op1=AL.add)
                if l == 5:
                    t5b = pfin.tile([64, F], f16, tag="T5b", name="t5b")
                    tiles['T5b'] = t5b
                    nc.sync.dma_start(t5b[:], r_t[64:128, :])
    nc.compile()
    _BASS_CACHE['nc'] = nc
    return nc


def _prep_inputs(x, idx_a, idx_b, ws):
    coef = _coef_tables(ws)
    rows = _row_tables(idx_a, idx_b)
    x = np.ascontiguousarray(x, dtype=np.float32)
    in_maps = []
    for core in range(NCORES):
        xs = x[B2 * core:B2 * core + B2].transpose(1, 2, 3, 0)  # [C,H,W,B2]
        in_maps.append({"xs": _crop_table(xs), "rows": rows, "coef": coef})
    return in_maps


def _assemble(core_outs, gamma):
    """core_outs: list of [64, F=(hh,ww,b)]; gamma [64] -> [16,64,900,1]."""
    full = np.stack([np.asarray(o, dtype=np.float32) for o in core_outs])
    full = full + gamma.astype(np.float32)[None, :, None]
    full = full.reshape(NCORES, K, P, B2)           # [core, k, p, b_local]
    full = full.transpose(0, 3, 1, 2).reshape(B, K, P, 1)
    return np.ascontiguousarray(full.astype(np.float32))


def kernel(x, idx_a, idx_b, w0, w1, w2, w3, w4, w5, w6):
    ws = [np.asarray(w, dtype=np.float32) for w in
          (w0, w1, w2, w3, w4, w5, w6)]
    x = np.asarray(x, dtype=np.float32)
    idx_a = np.asarray(idx_a, dtype=np.int32)
    idx_b = np.asarray(idx_b, dtype=np.int32)
    in_maps = _prep_inputs(x, idx_a, idx_b, ws)
    nc = _build_bass()
    from concourse.bass_utils import run_bass_kernel_spmd
    res = run_bass_kernel_spmd(nc, in_maps, core_ids=list(range(NCORES)))
    gamma = in_maps[0]["coef"][0:64, 4 * _NMIX]
    return _assemble([r["out"] for r in res.results], gamma)


def kernel_emulate(x, idx_a, idx_b, w0, w1, w2, w3, w4, w5, w6):
    """Pure-numpy emulation of the exact device schedule (debug aid)."""
    ws = [np.asarray(w, dtype=np.float32) for w in
          (w0, w1, w2, w3, w4, w5, w6)]
    in_maps = _prep_inputs(np.asarray(x, np.float32),
                           np.asarray(idx_a, np.int32),
                           np.asarray(idx_b, np.int32), ws)
    outs = [_emulate_core(m["xs"], m["rows"], m["coef"]) for m in in_maps]
    return _assemble(outs, in_maps[0]["coef"][0:64, 4 * _NMIX])


# revision 3
# speedup vs baseline: 1.1885x; 1.1885x over previous
"""Trainium2 Bass kernel for nn_LogicConv3d (differentiable-logic conv tree).

Problem (hardcoded): x [16,64,32,32] f32; idx_a/idx_b [64,900,64,3] i32;
w0..w6 [s,64,16] f32 (s = 64,32,16,8,4,2,1). Output [16,64,900,1] f32.

Math: per (kernel k, window p): gather 64 (a,b) leaf pairs from x, blend each
pair with soft-gate coefficients (softmax(w) @ GATE_M), then 6 more pairwise
tree levels.  mix(a,b) = c0 + c1*a + c2*b + c3*a*b.

v3 design (fp16 end-to-end, DVE 2x/4x perf modes):
 - F-sharding: core i handles batches (2i, 2i+1); pure SPMD across 8 cores.
 - Host builds a 576-row fp16 crop table XS[(c,ha,wa), 1920]: row = the
   30x30x2 (h,w,b-interleaved) crop of channel c at shift (ha,wa), compact in
   the first 1800 elements.  Leaf gathers are indirect DMAs with per-lane
   element offsets row*1920, fetching 1800 contiguous fp16 -> operands are
   step-1 fp16, which unlocks DVE packed modes.
 - scalar_tensor_tensor has NO DVE perf modes (1x only) so the mix avoids it:
     p = a*alpha + beta   (tensor_scalar 4x on DVE, or ACT activation)
     q = b * p            (tensor_tensor mult, 2x on DVE; some on Pool)
     u = a*gamma          (tensor_scalar / ACT)
     out = q + u          (tensor_tensor add, 2x on DVE)
   alpha = c3, beta = c2 - c3*Ta, gamma = c1 - c3*Tb where Ta/Tb are the
   children's additive-bias chain (bias folding; all multiplicative, safe).
   The per-node bias T = c2*Tb + c1*Ta - c3*Ta*Tb - c0 propagates on host in
   f64; the root bias is subtracted on host.
"""
import numpy as np

B, C, H, W = 16, 64, 32, 32
K = 64
RF = 3
DEPTH = 6
S = 64
PW = 30            # windows per axis
P = PW * PW        # 900
NCORES = 8
B2 = 2             # batches per core
F = P * B2         # free size (windows x batches) = 1800
NROW = C * RF * RF          # 576 crop-table rows
ROWE = 1920                 # crop-table row stride (elements)

# static engine assignment knobs (tuned against the HW trace)
P_ACT_EVERY = 4      # p-op on ACT when i % P_ACT_EVERY == 0, else DVE ts
Q_POOL_MOD, Q_POOL_LIM = 7, 2   # q-TT on Pool when i % MOD < LIM

GATE_M = np.array([
    [0, 0, 0, 0], [0, 0, 0, 1], [0, 1, 0, -1], [0, 1, 0, 0],
    [0, 0, 1, -1], [0, 0, 1, 0], [0, 1, 1, -2], [0, 1, 1, -1],
    [1, -1, -1, 1], [1, -1, -1, 2], [1, 0, -1, 0], [1, 0, -1, 1],
    [1, -1, 0, 0], [1, -1, 0, 1], [1, 0, 0, -1], [1, 0, 0, 0],
], dtype=np.float32)  # [16 gates, 4] -> c0,c1,c2,c3 = GATE_M.T @ softmax(w)


# ---------------------------------------------------------------------------
# static schedule: the merge-tree op list (DFS order keeps live tiles small)
# ---------------------------------------------------------------------------
def _build_schedule():
    """Each mix op: dict(level, key, lanes, base, node[lanes], kern[lanes]).
    L0 ops gather their own leaves; level l>=1 ops read T_{l-1}[2k],[2k+1]."""
    ops = []

    def emit(l, key):
        if l == 0:
            lanes = np.arange(128)
            ops.append(dict(level=0, key=key, lanes=128, base=0,
                            node=key + 32 * (lanes >> 6), kern=lanes & 63))
            return
        emit(l - 1, 2 * key)
        emit(l - 1, 2 * key + 1)
        lanes = np.arange(128)
        nbits_out = 6 - l
        ops.append(dict(level=l, key=key, lanes=128, base=0,
                        node=((lanes >> 6) << (nbits_out - 1)) + key,
                        kern=lanes & 63))

    emit(4, 0)
    emit(4, 1)
    # L5: one full op; node i5 = lane>>6 (a DMA then realigns the top half
    # to a base-0 tile for L6's equal-base inputs)
    lanes = np.arange(128)
    ops.append(dict(level=5, key=0, lanes=128, base=0,
                    node=lanes >> 6, kern=lanes & 63))
    lanes = np.arange(64)
    ops.append(dict(level=6, key=0, lanes=64, base=0,
                    node=np.zeros(64, np.int64), kern=lanes))
    return ops


_SCHED = _build_schedule()
_NMIX = len(_SCHED)          # 64
_NCOLS = 4 * _NMIX + 4       # + final root-bias column block


def _softmax_f32(w):
    w = w.astype(np.float64)
    m = w.max(-1, keepdims=True)
    e = np.exp(w - m)
    return e / e.sum(-1, keepdims=True)


def _coef_tables(ws):
    """ws = [w0..w6]. Returns coef matrix [128, _NCOLS] f32 with per-op scalar
    columns (alpha, beta, gamma, 0) and the final root-bias column
    (value to ADD on host: -T_root)."""
    cs = []
    for wl in ws:
        p = _softmax_f32(wl)                      # [s, K, 16] f64
        cs.append(np.einsum('skg,gj->skj', p, GATE_M.astype(np.float64)))
    # bias chain: T[l][node, kern] = delivered - true value at level-l output
    T = [None] * 7
    for l in range(7):
        c0, c1, c2, c3 = (cs[l][:, :, j] for j in range(4))
        if l == 0:
            Ta = np.zeros_like(c0)
            Tb = np.zeros_like(c0)
        else:
            Ta = T[l - 1][0::2]
            Tb = T[l - 1][1::2]
        T[l] = c2 * Tb + c1 * Ta - c3 * Ta * Tb - c0
    coef = np.zeros((128, _NCOLS), dtype=np.float64)
    for i, op in enumerate(_SCHED):
        l, node, kern = op['level'], op['node'], op['kern']
        rows = op['base'] + np.arange(op['lanes'])
        c = cs[l][node, kern]                     # [lanes, 4] = c0,c1,c2,c3
        if l == 0:
            Ta = np.zeros(op['lanes'])
            Tb = np.zeros(op['lanes'])
        else:
            Ta = T[l - 1][2 * node, kern]
            Tb = T[l - 1][2 * node + 1, kern]
        coef[rows, 4 * i + 0] = c[:, 3]                      # alpha = c3
        coef[rows, 4 * i + 1] = c[:, 2] - c[:, 3] * Ta       # beta
        coef[rows, 4 * i + 2] = c[:, 1] - c[:, 3] * Tb       # gamma
    coef[0:64, 4 * _NMIX] = -T[6][0, :]                      # final add
    return coef.astype(np.float32)


def _offset_tables(idx_a, idx_b):
    """Indirect-gather element-offset tables [128, 64] i32: col = 2*t + side.
    Offset = (c*9 + ha*3 + wa) * ROWE into the fp16 crop table."""
    offs = np.zeros((128, 64), dtype=np.int64)
    for op in _SCHED:
        if op['level'] != 0:
            continue
        t = op['key']
        for side, idx in ((0, idx_a), (1, idx_b)):
            ha = idx[op['kern'], 0, op['node'], 0].astype(np.int64)
            wa = idx[op['kern'], 0, op['node'], 1].astype(np.int64)
            ca = idx[op['kern'], 0, op['node'], 2].astype(np.int64)
            offs[:, 2 * t + side] = (ca * 9 + ha * 3 + wa) * ROWE
    return offs.astype(np.int32)


def _crop_table(xs):
    """xs: [C, H, W, B2] f32 b-interleaved slice -> XS [576, 1920] fp16."""
    XS = np.zeros((NROW, ROWE), dtype=np.float16)
    for ha in range(RF):
        for wa in range(RF):
            rows = np.arange(C) * 9 + ha * 3 + wa
            XS[rows, :F] = xs[:, ha:ha + PW, wa:wa + PW, :].reshape(
                C, F).astype(np.float16)
    return XS


# ---------------------------------------------------------------------------
# numpy emulator (mirrors the device schedule incl. fp16 rounding)
# ---------------------------------------------------------------------------
def _emulate_core(XS, offs, coef):
    """XS: [576,1920] fp16; offs: [128, 64] i32. Returns [64, F] f32."""
    f16 = np.float16
    XSf = XS.reshape(-1)
    tiles = {}
    for i, op in enumerate(_SCHED):
        l, key, n, base = op['level'], op['key'], op['lanes'], op['base']
        rws = base + np.arange(n)
        al = coef[rws, 4 * i + 0][:, None].astype(np.float32)
        be = coef[rws, 4 * i + 1][:, None].astype(np.float32)
        ga = coef[rws, 4 * i + 2][:, None].astype(np.float32)
        if l == 0:
            a = np.stack([XSf[o:o + F] for o in offs[:, 2 * key]])
            b = np.stack([XSf[o:o + F] for o in offs[:, 2 * key + 1]])
            a = a.astype(np.float32)
            b = b.astype(np.float32)
        elif l < 5:
            a = tiles[(l - 1, 2 * key)].astype(np.float32)
            b = tiles[(l - 1, 2 * key + 1)].astype(np.float32)
        elif l == 5:
            a = tiles[(4, 0)].astype(np.float32)
            b = tiles[(4, 1)].astype(np.float32)
        else:
            a = tiles['T5'][0:64].astype(np.float32)
            b = tiles['T5'][64:128].astype(np.float32)
        p = f16(a * al + be).astype(np.float32)
        q = f16(b * p).astype(np.float32)
        u = f16(a * ga).astype(np.float32)
        r = f16(q + u)
        if l == 5:
            tiles['T5'] = r
        else:
            tiles[(l, key)] = r
    return tiles[(6, 0)].astype(np.float32)


# ---------------------------------------------------------------------------
# Bass program (built once, cached)
# ---------------------------------------------------------------------------
_BASS_CACHE = {}


def _build_bass():
    if 'nc' in _BASS_CACHE:
        return _BASS_CACHE['nc']
    import concourse.bass as bass
    import concourse.mybir as mybir
    import concourse.tile as tile
    import concourse.bacc as bacc

    f32 = mybir.dt.float32
    f16 = mybir.dt.float16
    nc = bacc.Bacc("TRN2", target_bir_lowering=False, debug=False,
                   num_devices=NCORES)
    nxs = NROW * ROWE
    xs_d = nc.dram_tensor("xs", [nxs, 1], f16, kind="ExternalInput").ap()
    offs_d = nc.dram_tensor("offs", [128, 64], mybir.dt.int32,
                            kind="ExternalInput").ap()
    coef_d = nc.dram_tensor("coef", [128, _NCOLS], f32,
                            kind="ExternalInput").ap()
    out_d = nc.dram_tensor("out", [64, F], f16, kind="ExternalOutput").ap()

    AL = mybir.AluOpType
    ACTF = mybir.ActivationFunctionType

    with tile.TileContext(nc) as tc:
        with (
            tc.tile_pool(name="const", bufs=1) as pc,
            tc.tile_pool(name="ab", bufs=6) as pab,
            tc.tile_pool(name="lvl", bufs=2) as plv,
            tc.tile_pool(name="t0p", bufs=2) as pt0,
            tc.tile_pool(name="tmp", bufs=6) as ptmp,
            tc.tile_pool(name="fin", bufs=1) as pfin,
        ):
            offs_t = pc.tile([128, 64], mybir.dt.int32, tag="offs",
                             name="offs_t")
            nc.gpsimd.dma_start(offs_t[:], offs_d[:])
            coef_t = pc.tile([128, _NCOLS], f32, tag="coef", name="coef_t")
            nc.sync.dma_start(coef_t[:], coef_d[:])
            warm_t = pc.tile([1, 8], f32, tag="warm", name="warm_t")
            nc.scalar.activation(warm_t[:], coef_t[0:1, 0:8],
                                 ACTF.Identity, bias=0.0, scale=1.0)

            tiles = {}
            for i, op in enumerate(_SCHED):
                l, key, n, base = op['level'], op['key'], op['lanes'], op['base']
                sl = slice(base, base + n)
                al = coef_t[sl, 4 * i + 0:4 * i + 1]
                be = coef_t[sl, 4 * i + 1:4 * i + 2]
                ga = coef_t[sl, 4 * i + 2:4 * i + 3]
                if l == 0:
                    a_t = pab.tile([128, F], f16, tag="A", name="at")
                    b_t = pab.tile([128, F], f16, tag="B", name="bt")
                    for side, dst in ((0, a_t), (1, b_t)):
                        nc.gpsimd.indirect_dma_start(
                            out=dst[:], out_offset=None, in_=xs_d[:],
                            in_offset=bass.IndirectOffsetOnAxis(
                                ap=offs_t[:, 2 * key + side:
                                          2 * key + side + 1], axis=0))
                    a_ap, b_ap = a_t[:, :], b_t[:, :]
                elif l < 5:
                    a_ap = tiles[(l - 1, 2 * key)][:, :]
                    b_ap = tiles[(l - 1, 2 * key + 1)][:, :]
                elif l == 5:
                    a_ap = tiles[(4, 0)][:, :]
                    b_ap = tiles[(4, 1)][:, :]
                else:
                    a_ap = tiles['T5'][0:64, :]
                    b_ap = tiles['T5b'][:, :]

                # p = a*alpha + beta
                p_t = ptmp.tile([n, F], f16, tag="p", name="p")
                if i % P_ACT_EVERY == 0:
                    nc.scalar.activation(p_t[:, :], a_ap, ACTF.Identity,
                                         bias=be, scale=al)
                else:
                    nc.vector.tensor_scalar(
                        out=p_t[:, :], in0=a_ap, scalar1=al, scalar2=be,
                        op0=AL.mult, op1=AL.add)
                # u = a*gamma  (ACT engine)
                u_t = ptmp.tile([n, F], f16, tag="u", name="u")
                nc.scalar.activation(u_t[:, :], a_ap, ACTF.Identity,
                                     bias=0.0, scale=ga)
                # q = b * p  (in-place on p; DVE or Pool)
                q_eng = (nc.gpsimd if (i % Q_POOL_MOD) < Q_POOL_LIM
                         else nc.vector)
                q_eng.tensor_tensor(out=p_t[:, :], in0=b_ap, in1=p_t[:, :],
                                    op=AL.mult)
                # out = q + u
                if l == 5:
                    r_t = pfin.tile([128, F], f16, tag="T5", name="t5")
                    tiles['T5'] = r_t
                elif l == 6:
                    r_t = pfin.tile([64, F], f16, tag="T6", name="t6")
                else:
                    pool = pt0 if l == 0 else plv
                    r_t = pool.tile([128, F], f16, tag=f"T{l}",
                                    name=f"t{l}_{key}")
                    tiles[(l, key)] = r_t
                if l == 6:
                    # compute + store in h-halves so the store of half 0
                    # overlaps the add of half 1
                    for hh in (0, 1):
                        fs = slice(900 * hh, 900 * hh + 900)
                        nc.vector.tensor_tensor(
                            out=r_t[:, fs], in0=p_t[:, fs], in1=u_t[:, fs],
                            op=AL.add)
                        nc.sync.dma_start(out_d[:, fs], r_t[:, fs])
                    continue
                nc.vector.tensor_tensor(out=r_t[:, :], in0=p_t[:, :],
                                        in1=u_t[:, :], op=AL.add)
                if l == 5:
                    t5b = pfin.tile([64, F], f16, tag="T5b", name="t5b")
                    tiles['T5b'] = t5b
                    nc.sync.dma_start(t5b[:], r_t[64:128, :])
    nc.compile()
    _BASS_CACHE['nc'] = nc
    return nc


def _prep_inputs(x, idx_a, idx_b, ws):
    coef = _coef_tables(ws)
    offs = _offset_tables(idx_a, idx_b)
    x = np.ascontiguousarray(x, dtype=np.float32)
    in_maps = []
    for core in range(NCORES):
        xs = x[B2 * core:B2 * core + B2].transpose(1, 2, 3, 0)  # [C,H,W,B2]
        in_maps.append({"xs": _crop_table(xs).reshape(-1, 1),
                        "offs": offs, "coef": coef})
    return in_maps


def _assemble(core_outs, gamma):
    """core_outs: list of [64, F=(hh,ww,b)]; gamma [64] -> [16,64,900,1]."""
    full = np.stack([np.asarray(o, dtype=np.float32) for o in core_outs])
    full = full + gamma.astype(np.float32)[None, :, None]
    full = full.reshape(NCORES, K, P, B2)           # [core, k, p, b_local]
    full = full.transpose(0, 3, 1, 2).reshape(B, K, P, 1)
    return np.ascontiguousarray(full.astype(np.float32))


def kernel(x, idx_a, idx_b, w0, w1, w2, w3, w4, w5, w6):
    ws = [np.asarray(w, dtype=np.float32) for w in
          (w0, w1, w2, w3, w4, w5, w6)]
    x = np.asarray(x, dtype=np.float32)
    idx_a = np.asarray(idx_a, dtype=np.int32)
    idx_b = np.asarray(idx_b, dtype=np.int32)
    in_maps = _prep_inputs(x, idx_a, idx_b, ws)
    nc = _build_bass()
    from concourse.bass_utils import run_bass_kernel_spmd
    res = run_bass_kernel_spmd(nc, in_maps, core_ids=list(range(NCORES)))
    gamma = in_maps[0]["coef"][0:64, 4 * _NMIX]
    return _assemble([r["out"] for r in res.results], gamma)


def kernel_emulate(x, idx_a, idx_b, w0, w1, w2, w3, w4, w5, w6):
    """Pure-numpy emulation of the exact device schedule (debug aid)."""
    ws = [np.asarray(w, dtype=np.float32) for w in
          (w0, w1, w2, w3, w4, w5, w6)]
    in_maps = _prep_inputs(np.asarray(x, np.float32),
                           np.asarray(idx_a, np.int32),
                           np.asarray(idx_b, np.int32), ws)
    outs = [_emulate_core(m["xs"].reshape(NROW, ROWE), m["offs"], m["coef"])
            for m in in_maps]
    return _assemble(outs, in_maps[0]["coef"][0:64, 4 * _NMIX])
